# revision 39
# baseline (speedup 1.0000x reference)
"""TRN2 Bass kernel for nn_CausalSelfAttention_63058709840004.

Sharding: tensor-parallel over heads (2 groups x 3 heads) x 4 causal query
shards = 8 cores. Each core computes K,V for its 3 heads over the full
sequence (replicated within the group), Q for its 1024 query rows (two
512-row chunks at s*512 and (s+4)*512), runs causal attention, and a partial
c_proj; the host sums the two head-group partials per row.

v2: bf16 operands on every matmul input (PE stays 1 cyc/row at >=256 free,
HBM DMA bytes and SBUF footprint halve, DVE copies of bf16 PSUM get the 2x
perf mode), x/ve/weights/tables shipped as bf16, ve upcast to f32 in the
SWDGE, y-normalize fused into a single DVE mul (no separate PSUM copy),
c_proj interleaved per query chunk so its matmuls overlap the second chunk's
attention. Softmax skips max-subtraction (|scores| <= 15.4 bounded by
Cauchy-Schwarz after rms-norm, safe in fp32 PSUM). Measured on HW (slope of
a reps-loop build): ~243us steady-state vs ~285us for the f32r baseline.

Rejected on HW evidence: softmax-denominator accumulation on GpSimd/Pool
(TensorTensor there is far slower than the cost model claims: +120us),
rms-scale-via-diag matmul + Pool squares (+20us), paired 2-bank [128,1024]
exps (no gain). GPSIMD cannot access PSUM at all (BIR verifier).
"""
import contextlib

import numpy as np
import ml_dtypes

import concourse.bass as bass
import concourse.bacc as bacc
import concourse.mybir as mybir
import concourse.tile as tile
from concourse.bass_utils import run_bass_kernel_spmd

T, DIM, H, D = 4096, 768, 6, 128
HPG = 3  # heads per group
GDIM = HPG * D  # 384
ATTN_SCALE = 0.12
EPS = 1.1920929e-07
NT = T // 128  # 32 t-tiles
NQ = 1024 // 128  # 8 q-tiles per core
F32 = mybir.dt.float32
F32R = mybir.dt.float32r
BF16 = mybir.dt.bfloat16
U32 = mybir.dt.uint32
EXP = mybir.ActivationFunctionType.Exp
SQRT = mybir.ActivationFunctionType.Sqrt
SQUARE = mybir.ActivationFunctionType.Square
MULT = mybir.AluOpType.mult
ADD = mybir.AluOpType.add
BF = ml_dtypes.bfloat16

_CACHE = {}


def _rotary(nc, pool, nat, cos_b, sin_p, sin_n, nh):
    """In-place rotary on nat [128, nh, 128] bf16. Rotates dim pairs
    (i, 64+i) for i in 0..31 (freqs 32..63 are zero -> identity)."""
    x1 = nat[:, :, 0:32]
    x2 = nat[:, :, 64:96]
    ta = pool.tile([128, nh, 32], BF16, tag="rot_ta")
    tb = pool.tile([128, nh, 32], BF16, tag="rot_tb")
    ua = pool.tile([128, nh, 32], BF16, tag="rot_ua")
    ub = pool.tile([128, nh, 32], BF16, tag="rot_ub")
    nc.vector.tensor_mul(out=ta[:], in0=x2, in1=sin_p)  # x2*sin
    nc.vector.tensor_mul(out=tb[:], in0=x1, in1=sin_n)  # -x1*sin
    nc.vector.tensor_mul(out=ua[:], in0=x1, in1=cos_b)  # x1*cos
    nc.vector.tensor_mul(out=ub[:], in0=x2, in1=cos_b)  # x2*cos
    nc.vector.tensor_add(out=x1, in0=ua[:], in1=ta[:])  # y1 = x1*c + x2*s
    nc.vector.tensor_add(out=x2, in0=ub[:], in1=tb[:])  # y2 = x2*c - x1*s


def build_nc(variant=None, loop_reps=False):
    nc = bacc.Bacc(None, target_bir_lowering=False)

    # ---- DRAM tensors (per-core inputs prepared by the host) ----
    if loop_reps:
        reps_t = nc.dram_tensor("reps", [1, 1], U32, kind="ExternalInput")
    xTt = nc.dram_tensor("xTt", [NT // 2, 128, 6, 2, 128], BF16, kind="ExternalInput")
    xqTt = nc.dram_tensor("xqTt", [NQ // 2, 128, 6, 2, 128], BF16, kind="ExternalInput")
    wkv = nc.dram_tensor("wkv", [128, 6, 2 * GDIM], BF16, kind="ExternalInput")
    wq = nc.dram_tensor("wq", [128, 6, GDIM], BF16, kind="ExternalInput")
    vek = nc.dram_tensor("vek", [NT // 2, 128, 2, GDIM], BF16, kind="ExternalInput")
    cosk = nc.dram_tensor("cosk", [128, NT, 32], BF16, kind="ExternalInput")
    sinkpm = nc.dram_tensor("sinkpm", [128, NT, 64], BF16, kind="ExternalInput")
    cosq = nc.dram_tensor("cosq", [128, NQ, 32], BF16, kind="ExternalInput")
    sinqpm = nc.dram_tensor("sinqpm", [128, NQ, 64], BF16, kind="ExternalInput")
    cprojT = nc.dram_tensor("cprojT", [128, HPG, DIM], BF16, kind="ExternalInput")
    ident_in = nc.dram_tensor("ident", [128, 128], BF16, kind="ExternalInput")
    ones_col_in = nc.dram_tensor("ones_col", [128, 1], BF16, kind="ExternalInput")
    svar_t = nc.dram_tensor("svar", [1, 1], U32, kind="ExternalInput")
    y_out = nc.dram_tensor("y", [1024, DIM], F32, kind="ExternalOutput")

    with tile.TileContext(nc) as tc:
        # core-variant register (s = core % 4)
        tmp = nc.alloc_registers("tmp_svar", mybir.ALL_ENGINES)
        nc.regs_load(tmp, svar_t[0:1, 0:1])
        sv = nc.snap(tmp, donate=True, min_val=0, max_val=3)

        if loop_reps:
            # timing-only build: run the whole body `reps` times so exec time
            # can be measured as a slope across trip counts on one NEFF
            rtmp = nc.alloc_registers("tmp_reps", mybir.ALL_ENGINES)
            nc.regs_load(rtmp, reps_t[0:1, 0:1])
            rv = nc.snap(rtmp, donate=True, min_val=0, max_val=256)
            loop_cm = tc.For_i(0, rv)
        else:
            loop_cm = contextlib.nullcontext()

        with loop_cm, tc.tile_pool(name="res", bufs=1) as res:
            KT = res.tile([128, HPG, T], BF16, tag="KT")
            Vn = res.tile([128, NT, GDIM], BF16, tag="Vn")
            QT = res.tile([128, HPG, 1024], BF16, tag="QT")
            Ysb = res.tile([128, HPG, 1024], BF16, tag="Ysb")
            cproj_sb = res.tile([128, HPG, DIM], BF16, tag="cproj")
            ident = res.tile([128, 128], BF16, tag="ident")
            ones_col = res.tile([128, 1], BF16, tag="ones_col")
            nc.gpsimd.dma_start(ident[:], ident_in[:])
            nc.gpsimd.dma_start(ones_col[:], ones_col_in[:])
            eps_k = res.tile([128, 1], F32, tag="eps_k")
            eps_q = res.tile([128, 1], F32, tag="eps_q")
            nc.gpsimd.memset(eps_k[:], EPS)
            nc.gpsimd.memset(eps_q[:], EPS / (ATTN_SCALE * ATTN_SCALE))

            # ================= Phase A/B: projections =================
            with (
                tc.tile_pool(name="wp", bufs=1) as wp,
                tc.tile_pool(name="ap", bufs=3) as ap,
                tc.tile_pool(name="st", bufs=3) as st,
                tc.tile_pool(name="rot", bufs=1) as rot,
                tc.tile_pool(name="pp", bufs=3, space="PSUM") as pp,
                tc.tile_pool(name="pt", bufs=2, space="PSUM") as pt,
            ):
                wkv_sb = wp.tile([128, 6, 2 * GDIM], BF16, tag="wkv")
                wq_sb = wp.tile([128, 6, GDIM], BF16, tag="wq")
                cosk_sb = wp.tile([128, NT, 32], BF16, tag="cosk")
                sinkpm_sb = wp.tile([128, NT, 64], BF16, tag="sinkpm")
                cosq_sb = wp.tile([128, NQ, 32], BF16, tag="cosq")
                sinqpm_sb = wp.tile([128, NQ, 64], BF16, tag="sinqpm")
                for md in range(6):
                    nc.scalar.dma_start(wkv_sb[:, md], wkv[:, md])
                    nc.scalar.dma_start(wq_sb[:, md], wq[:, md])
                nc.gpsimd.dma_start(cosk_sb[:], cosk[:])
                nc.gpsimd.dma_start(sinkpm_sb[:], sinkpm[:])
                nc.gpsimd.dma_start(cosq_sb[:], cosq[:])
                nc.gpsimd.dma_start(sinqpm_sb[:], sinqpm[:])

                def norm_rot_transpose(ps_tile, dest, ti, eps_ap, sc, cos_sb, sin_sb):
                    """rms-norm + rotary + transpose [128,384] PSUM f32 ->
                    dest[:, :, ti*128:(ti+1)*128] bf16."""
                    # sum-of-squares per head on Act (reads PSUM directly)
                    ssq = ap.tile([128, HPG], F32, tag="ssq")
                    scratch = ap.tile([128, D], F32, tag="scratch")
                    for h in range(HPG):
                        nc.scalar.activation(
                            scratch[:], ps_tile[:, h * D : (h + 1) * D],
                            SQUARE, accum_out=ssq[:, h : h + 1],
                        )
                    bsc = ap.tile([128, HPG], F32, tag="bsc")
                    nc.scalar.activation(bsc[:], ssq[:], SQRT, bias=eps_ap, scale=sc)
                    nc.vector.reciprocal(bsc[:], bsc[:])
                    # scale applies during PSUM->SBUF evacuation (DVE mul)
                    nat = ap.tile([128, HPG, D], BF16, tag="knat")
                    nc.vector.tensor_mul(
                        out=nat[:],
                        in0=ps_tile[:].rearrange("p (h d) -> p h d", d=D),
                        in1=bsc[:, :, None].to_broadcast((128, HPG, D)),
                    )
                    _rotary(
                        nc, rot, nat,
                        cos_sb[:, ti, None, :].to_broadcast((128, HPG, 32)),
                        sin_sb[:, ti, None, 0:32].to_broadcast((128, HPG, 32)),
                        sin_sb[:, ti, None, 32:64].to_broadcast((128, HPG, 32)),
                        HPG,
                    )
                    tr = pt.tile([128, GDIM], BF16, tag="tr")
                    for h in range(HPG):
                        nc.tensor.transpose(
                            tr[:, h * D : (h + 1) * D], nat[:, h], ident[:]
                        )
                    # bf16 PSUM->SBUF evacuation (DVE 2x mode)
                    nc.vector.tensor_copy(
                        dest[:, :, ti * 128 : (ti + 1) * 128],
                        tr[:].rearrange("p (h d) -> p h d", d=D),
                    )

                # ---- Phase A: K,V over full sequence (two t-tiles per iter;
                # xt via SP HWDGE, ve via gpsimd SWDGE to spread dispatch) ----
                for tp in range(NT // 2):
                    xt2 = st.tile([128, 6, 2, 128], BF16, tag="xt")
                    nc.sync.dma_start(xt2[:], xTt[tp])
                    # f32 SBUF via casting SWDGE so the DVE add reads f32+f32
                    vet2 = st.tile([128, 2, GDIM], F32, tag="vet")
                    nc.gpsimd.dma_start(vet2[:], vek[tp])
                    for u in range(2):
                        ti = 2 * tp + u
                        k_ps = pp.tile([128, GDIM], F32, tag="k_ps")
                        v_ps = pp.tile([128, GDIM], F32, tag="v_ps")
                        for md in range(6):
                            nc.tensor.matmul(
                                k_ps[:], xt2[:, md, u], wkv_sb[:, md, 0:GDIM],
                                start=(md == 0), stop=(md == 5), skip_group_check=True,
                            )
                            nc.tensor.matmul(
                                v_ps[:], xt2[:, md, u], wkv_sb[:, md, GDIM : 2 * GDIM],
                                start=(md == 0), stop=(md == 5), skip_group_check=True,
                            )
                        # V: add pre-scaled ve (DVE; Pool can't read PSUM)
                        nc.vector.tensor_add(
                            out=Vn[:, ti, :], in0=v_ps[:], in1=vet2[:, u]
                        )
                        norm_rot_transpose(
                            k_ps, KT, ti, eps_k[:], 1.0 / D, cosk_sb, sinkpm_sb
                        )

                # ---- Phase B: Q over this core's 1024 rows ----
                for tp in range(NQ // 2):
                    xt2 = st.tile([128, 6, 2, 128], BF16, tag="xt")
                    nc.sync.dma_start(xt2[:], xqTt[tp])
                    for u in range(2):
                        ti = 2 * tp + u
                        q_ps = pp.tile([128, GDIM], F32, tag="k_ps")
                        for md in range(6):
                            nc.tensor.matmul(
                                q_ps[:], xt2[:, md, u], wq_sb[:, md],
                                start=(md == 0), stop=(md == 5), skip_group_check=True,
                            )
                        # a = ATTN_SCALE / sqrt(mean+eps) = 1/sqrt((m/D+eps)/s^2)
                        # folded via eps_q bias and 1/(D*s^2) scale
                        s2 = ATTN_SCALE * ATTN_SCALE
                        norm_rot_transpose(
                            q_ps, QT, ti, eps_q[:], 1.0 / (D * s2), cosq_sb, sinqpm_sb
                        )

            nc.gpsimd.dma_start(cproj_sb[:], cprojT[:])

            # ================= Phase C: attention (per-core variant) ======
            def attention(s):
                with (
                    tc.tile_pool(name=f"ep{s}", bufs=6) as ep,
                    tc.tile_pool(name=f"rp{s}", bufs=2) as rp,
                    tc.tile_pool(name=f"psS{s}", bufs=4, space="PSUM") as psS,
                    tc.tile_pool(name=f"psY{s}", bufs=2, space="PSUM") as psY,
                    tc.tile_pool(name=f"psD{s}", bufs=2, space="PSUM") as psD,
                ):
                    # paired causal chunks (s, 7-s): 36 key-tiles total on every core
                    chunks = [(0, 4 * s + 4), (512, 32 - 4 * s)]
                    for qoff, nk in chunks:
                        for h in range(HPG):
                            y_ps = psY.tile([128, 512], F32, tag="y")
                            d_ps = psD.tile([1, 512], F32, tag="d")
                            for k in range(nk):
                                s_ps = psS.tile([128, 512], F32, tag="s")
                                nc.tensor.matmul(
                                    s_ps[:],
                                    KT[:, h, k * 128 : (k + 1) * 128],
                                    QT[:, h, qoff : qoff + 512],
                                    start=True, stop=True, skip_group_check=True,
                                )
                                E = ep.tile([128, 512], BF16, tag="E")
                                nc.scalar.activation(E[:], s_ps[:], EXP)
                                i = k - (nk - 4)
                                if i >= 0:
                                    # zero E where key row 128*i+r > query col
                                    nc.gpsimd.affine_select(
                                        out=E[:], in_=E[:],
                                        compare_op=mybir.AluOpType.is_ge,
                                        fill=0.0, base=-128 * i,
                                        pattern=[[1, 512]], channel_multiplier=-1,
                                    )
                                nc.tensor.matmul(
                                    d_ps[:], ones_col[:], E[:],
                                    start=(k == 0), stop=(k == nk - 1),
                                    skip_group_check=True,
                                )
                                nc.tensor.matmul(
                                    y_ps[:], Vn[:, k, h * D : (h + 1) * D], E[:],
                                    start=(k == 0), stop=(k == nk - 1),
                                    skip_group_check=True,
                                )
                            recip = rp.tile([1, 512], F32R, tag="recip")
                            with nc.allow_low_precision(
                                reason="1/denom as f32r; ~1e-4 uniform scale wobble"
                            ):
                                nc.vector.reciprocal(recip[:], d_ps[:])
                            bc = rp.tile([128, 512], F32R, tag="bc")
                            nc.gpsimd.partition_broadcast(bc[:], recip[0:1, :])
                            # fused copy+normalize: Ysb = y_ps * (1/denom)
                            nc.vector.tensor_mul(
                                out=Ysb[:, h, qoff : qoff + 512],
                                in0=y_ps[:], in1=bc[:],
                            )
                with (
                    tc.tile_pool(name=f"op{s}", bufs=3) as op,
                    tc.tile_pool(name=f"psO{s}", bufs=3, space="PSUM") as psO,
                ):
                    for m in range(NQ):
                        o_sb = op.tile([128, DIM], F32, tag="o_sb")
                        for oc in range(2):
                            o_ps = psO.tile([128, GDIM], F32, tag="o_ps")
                            for h in range(HPG):
                                nc.tensor.matmul(
                                    o_ps[:],
                                    Ysb[:, h, m * 128 : (m + 1) * 128],
                                    cproj_sb[:, h, oc * GDIM : (oc + 1) * GDIM],
                                    start=(h == 0), stop=(h == 2),
                                    skip_group_check=True,
                                )
                            nc.vector.tensor_copy(
                                o_sb[:, oc * GDIM : (oc + 1) * GDIM], o_ps[:]
                            )
                        nc.sync.dma_start(y_out[m * 128 : (m + 1) * 128, :], o_sb[:])

            if variant is not None:
                attention(variant)
            else:
                with tc.If(sv == 0) as c0:
                    attention(0)
                with c0.Else():
                    with tc.If(sv == 1) as c1:
                        attention(1)
                    with c1.Else():
                        with tc.If(sv == 2) as c2:
                            attention(2)
                        with c2.Else():
                            attention(3)

    nc.finalize()
    return nc


def _host_prep(x, ve, qkv_w, lambdas, c_proj_w):
    """Build the 8 per-core input maps."""
    x2d = np.ascontiguousarray(x.reshape(T, DIM), dtype=np.float32)
    xT = np.ascontiguousarray(x2d.T)
    ve2 = ve.reshape(T, H, D).astype(np.float32)
    lam0, lam1 = float(lambdas[0]), float(lambdas[1])
    wq_all, wk_all, wv_all = qkv_w[0], qkv_w[1], qkv_w[2]  # [768, 768] each

    t = np.arange(T, dtype=np.float32)
    af = (1.0 / 1024.0) ** np.linspace(0.0, 1.0, 32, dtype=np.float32)
    theta = t[:, None] * af[None, :]
    cos_t = np.cos(theta).astype(np.float32)  # [T, 32]
    sin_t = np.sin(theta).astype(np.float32)
    sin_pm = np.concatenate([sin_t, -sin_t], axis=1)  # [T, 64]

    ident = np.eye(128, dtype=np.float32)
    ones_col = np.ones((128, 1), dtype=np.float32)

    def pack_xT(m):  # [768, t] -> [t/256, 128, 6, 2, 128]
        t = m.shape[1]
        return np.ascontiguousarray(
            m.reshape(6, 128, t // 256, 2, 128).transpose(2, 1, 0, 3, 4)
        ).astype(BF)

    def pack_rows(m):  # [t, d] -> [t/256, 128, 2, d]
        t, d = m.shape
        return np.ascontiguousarray(
            m.reshape(t // 256, 2, 128, d).transpose(0, 2, 1, 3)
        ).astype(BF)

    def pack_tab(m):  # [t, c] -> [128, t/128, c]
        t, c = m.shape
        return np.ascontiguousarray(
            m.reshape(t // 128, 128, c).transpose(1, 0, 2)
        ).astype(BF)

    xT_packed = pack_xT(xT)
    cosk_p = pack_tab(cos_t)
    sinkpm_p = pack_tab(sin_pm)
    in_maps = []
    for c in range(8):
        g, s = divmod(c, 4)
        hsl = slice(g * GDIM, (g + 1) * GDIM)
        qrows = np.r_[512 * s : 512 * (s + 1), 512 * (7 - s) : 512 * (8 - s)]
        wkv = np.concatenate([wk_all[hsl], lam0 * wv_all[hsl]], axis=0)  # [768, 768]
        in_maps.append(
            {
                "xTt": xT_packed,
                "xqTt": pack_xT(np.ascontiguousarray(xT[:, qrows])),
                "wkv": np.ascontiguousarray(
                    wkv.T.astype(np.float32).reshape(6, 128, 768).transpose(1, 0, 2)
                ).astype(BF),
                "wq": np.ascontiguousarray(
                    wq_all[hsl].T.astype(np.float32).reshape(6, 128, GDIM).transpose(1, 0, 2)
                ).astype(BF),
                "vek": pack_rows(
                    (lam1 * ve2[:, g * HPG : (g + 1) * HPG, :]).reshape(T, GDIM)
                ),
                "cosk": cosk_p,
                "sinkpm": sinkpm_p,
                "cosq": pack_tab(np.ascontiguousarray(cos_t[qrows])),
                "sinqpm": pack_tab(np.ascontiguousarray(sin_pm[qrows])),
                "cprojT": np.ascontiguousarray(
                    c_proj_w[:, hsl].T.astype(np.float32).reshape(HPG, 128, DIM).transpose(1, 0, 2)
                ).astype(BF),
                "ident": ident.astype(BF),
                "ones_col": ones_col.astype(BF),
                "svar": np.array([[s]], dtype=np.uint32),
            }
        )
    return in_maps


def run(inputs, **run_kwargs):
    if "nc" not in _CACHE:
        _CACHE["nc"] = build_nc()
    nc = _CACHE["nc"]
    in_maps = _host_prep(
        inputs["x"], inputs["ve"], inputs["qkv_w"], inputs["lambdas"], inputs["c_proj_w"]
    )
    res = run_bass_kernel_spmd(nc, in_maps, core_ids=list(range(8)), **run_kwargs)
    out = np.zeros((T, DIM), dtype=np.float32)
    for c, r in enumerate(res.results):
        s = c % 4
        y = r["y"]
        out[512 * s : 512 * (s + 1)] += y[:512]
        out[512 * (7 - s) : 512 * (8 - s)] += y[512:]
    return out.reshape(1, T, DIM), res


def kernel(**inputs):
    out, _ = run(inputs)
    return out
